# revision 27
# baseline (speedup 1.0000x reference)
"""AdaptiveSampler Trainium2 kernel: batch-parallel frame gather across 8 NeuronCores.

Reference semantics: out[b, j*4+g] = x[b, ceil(mu[b,j,g])] (zero frame when the
sampled index falls outside [0, T-1]), with
  mu[b,j,g] = (dt[b,j]*31.5 + 31.5) + (g - 1.5) * ((64/3 - 1)*delta_t[b,j] + 1).

Strategy: pure data parallelism over batch (4 samples/core). The sampled frame
indices are computed host-side (bit-identical to the jax reference, on jax-CPU)
and shipped as a tiny int32 tensor. On-device the kernel is an indirect-DMA
gather (HBM->SBUF) + indirect scatter (SBUF->HBM); out-of-range anchors are
skipped on both sides (descriptor-level skip via bounds_check), so zero frames
come from the pre-zeroed output buffer and cost no HBM traffic.

Hardware facts baked in (measured on trn2 via neuron-profile):
- indirect-DMA row size is a 16-bit byte field -> frames are split into SUB=4
  subrows of 37632 B (SUB=3 / 50176 B also fits but adds 176 OOB dummy
  packets per core and measures ~6 us slower).
- partition-sliced indirect DMAs fail at runtime -> every DMA spans all 128
  partitions; the output subrows live in 2 column blocks.
- descriptor -> SDMA-engine mapping is engine(p) = ((p//4) % 8)*2 + p//64;
  OOB slots cost only a 4-byte dummy packet. The host assigns subrows to SBUF
  slots so all 16 engines carry equal byte counts, and bin-packs batches onto
  cores so all 8 cores carry equal valid-frame counts.
"""

import os

import numpy as np

import concourse.bass as bass
import concourse.mybir as mybir
from concourse.bass_utils import run_bass_kernel_spmd

B, T, C, H, W = 32, 64, 3, 112, 112
AOT = 4                      # output frames per anchor; 4 anchors
NCORES = 8
BL = B // NCORES             # local batches per core
CHW = C * H * W              # 37632 floats per frame
SUB = 4                      # subrows per frame (row bytes must be <= 65535)
SUBLEN = CHW // SUB          # 9408 floats = 37632 B per subrow
NROWS_IN = BL * T * SUB      # 1024 source subrows per core
FRAMES_OUT = BL * AOT * AOT  # 64 output frames per core
NROWS_OUT = FRAMES_OUT * SUB # 256 output subrows per core
NPART = 128
NBLK = 2                     # column blocks in SBUF (256 slots = 256 subrows)
OOB = 1 << 30

TRACE = False
CROSS_BALANCE = True
RUN_KWARGS = {}
LAST_RESULT = None

_graph_cache = {}


def _build_graph():
    nc = bass.Bass()
    xz = nc.declare_dram_parameter("xz", [NROWS_IN, SUBLEN], mybir.dt.float32, isOutput=False)
    idx = nc.declare_dram_parameter("idx", [NPART, 2 * NBLK], mybir.dt.int32, isOutput=False)
    out = nc.declare_dram_parameter("out", [NROWS_OUT, SUBLEN], mybir.dt.float32, isOutput=True)

    with (
        nc.sbuf_tensor("buf", [NPART, NBLK * SUBLEN], mybir.dt.float32) as buf,
        nc.sbuf_tensor("idxs", [NPART, 2 * NBLK], mybir.dt.int32) as idxs,
        nc.semaphore("s_idx") as s_idx,
        nc.semaphore("s_g") as s_g,
        nc.semaphore("s_s") as s_s,
        nc.Block() as block,
    ):
        @block.sync
        def _(sync):
            sync.dma_start(out=idxs[:, :], in_=idx[:, :]).then_inc(s_idx, 16)

        @block.gpsimd
        def _(gpsimd):
            rb_in = gpsimd.to_reg(NROWS_IN - 1)
            rb_out = gpsimd.to_reg(NROWS_OUT - 1)
            first = [True]

            def fuse_wait(ins):
                if first[0]:
                    ins._wait_ge(s_idx, 16)
                    first[0] = False
                return ins

            def gather(blk):
                fuse_wait(gpsimd.indirect_dma_start(
                    out=buf[:, blk * SUBLEN:(blk + 1) * SUBLEN],
                    out_offset=None,
                    in_=xz[:, :],
                    in_offset=bass.IndirectOffsetOnAxis(ap=idxs[:, 2 * blk:2 * blk + 1], axis=0),
                    bounds_check=rb_in,
                    oob_is_err=False,
                )).then_inc(s_g, 16)

            def scatter(blk):
                gpsimd.indirect_dma_start(
                    out=out[:, :],
                    out_offset=bass.IndirectOffsetOnAxis(ap=idxs[:, 2 * blk + 1:2 * blk + 2], axis=0),
                    in_=buf[:, blk * SUBLEN:(blk + 1) * SUBLEN],
                    in_offset=None,
                    bounds_check=rb_out,
                    oob_is_err=False,
                )._wait_ge(s_g, 16 * (blk + 1)).then_inc(s_s, 16)

            for blk in range(NBLK):
                gather(blk)
            for blk in range(NBLK):
                scatter(blk)
            gpsimd.wait_ge(s_s, 16 * NBLK)

    return nc


def _get_graph():
    if "nc" not in _graph_cache:
        _graph_cache["nc"] = _build_graph()
    return _graph_cache["nc"]


def _frame_indices(dt, delta_t):
    """ceil(mu) per (b, j, g), bit-identical to the jax reference (on jax-CPU)."""
    import jax
    import jax.numpy as jnp

    with jax.default_device(jax.devices("cpu")[0]):
        dtj = jnp.asarray(np.asarray(dt, dtype=np.float32))
        dlj = jnp.asarray(np.asarray(delta_t, dtype=np.float32))
        anchor_t = (T - 1) / 2.0
        dts = dtj * anchor_t + anchor_t
        deltas = (T / (AOT - 1) - 1.0) * dlj + 1.0
        grid = jnp.arange(AOT, dtype=jnp.float32)
        mu = dts[:, :, None] + (grid[None, None, :] - (AOT - 1) / 2.0) * deltas[:, :, None]
        idxf = np.asarray(jnp.ceil(mu))  # [B, AOT, AOT] float32
    valid = (idxf >= 0) & (idxf <= T - 1)
    t_idx = np.where(valid, idxf, 0).astype(np.int64)
    return t_idx.reshape(B, AOT * AOT), valid.reshape(B, AOT * AOT)


def kernel(x, dt, delta_t):
    global LAST_RESULT
    x = np.ascontiguousarray(np.asarray(x), dtype=np.float32)
    t_flat, v_flat = _frame_indices(dt, delta_t)

    # cross-core load balance: batches differ in valid-frame count, so greedily
    # bin-pack them (4 per core) to equalize per-core DMA bytes
    if CROSS_BALANCE:
        vcnt = v_flat.sum(axis=1)
        loads = [0] * NCORES
        packs = [[] for _ in range(NCORES)]
        for b in sorted(range(B), key=lambda b: -vcnt[b]):
            m = min((m for m in range(NCORES) if len(packs[m]) < BL), key=lambda m: loads[m])
            packs[m].append(b)
            loads[m] += vcnt[b]
    else:
        packs = [list(range(m * BL, (m + 1) * BL)) for m in range(NCORES)]

    # valid output subrows, balanced round-robin across the 16 SDMA engines
    # (and across the 2 column-block DMAs within an engine)
    q = np.arange(FRAMES_OUT)
    bl = q // (AOT * AOT)
    f = q % (AOT * AOT)

    in_maps = []
    for m in range(NCORES):
        batches = np.asarray(packs[m])
        xs = np.ascontiguousarray(x[batches]).reshape(NROWS_IN, SUBLEN)
        b = batches[bl]
        okq = v_flat[b, f]
        vq = q[okq]
        # per-frame subrows, contiguous subrows of one frame spread over engines
        dst = (SUB * vq[:, None] + np.arange(SUB)[None, :]).ravel()
        tsrc = t_flat[b, f][okq]
        src = (SUB * (bl[okq] * T + tsrc)[:, None] + np.arange(SUB)[None, :]).ravel()
        n = len(dst)
        i = np.arange(n)
        eng = i % 16
        rank = i // 16                    # slot rank within engine (0..15)
        blk_a = rank % NBLK
        jj = rank // NBLK                 # which of the engine's 8 partitions
        part_a = 4 * (eng // 2) + 64 * (eng % 2) + 32 * (jj // 4) + jj % 4
        idx_np = np.full((NPART, 2 * NBLK), OOB, np.int32)
        idx_np[part_a, 2 * blk_a] = src
        idx_np[part_a, 2 * blk_a + 1] = dst
        in_maps.append({"xz": xs, "idx": idx_np})

    if TRACE:
        os.environ.pop("BASS_NEVER_TRACE", None)
    else:
        # an inherited BASS_TRACE=1 would route run_bass_kernel_spmd through
        # the NTFF-profiling path, which this image can't always serve
        os.environ["BASS_NEVER_TRACE"] = "1"

    nc = _get_graph()
    last_err = None
    for attempt in range(3):
        try:
            LAST_RESULT = run_bass_kernel_spmd(
                nc, in_maps, core_ids=list(range(NCORES)), trace=TRACE, **RUN_KWARGS
            )
            break
        except Exception as e:  # transient NRT_EXEC_UNIT_UNRECOVERABLE etc.
            last_err = e
            import time
            time.sleep(5 * (attempt + 1))
    else:
        raise last_err
    out_full = np.empty((B, AOT * AOT, C, H, W), np.float32)
    for m, r in enumerate(LAST_RESULT.results):
        out_full[np.asarray(packs[m])] = r["out"].reshape(BL, AOT * AOT, C, H, W)
    return out_full
